# revision 4
# baseline (speedup 1.0000x reference)
"""Channel-attention block kernel for Trainium2 — V2: fp8 DoubleRow MM2 +
flat cross-batch software pipeline.

Per batch t (C=512, N=4096):
    aff = q qT (bf16, upper 10/16 block-triangle + 6 PE-transpose recons)
    attn = exp(rowmin - aff) * gamma/Z  -> fp8e4
    out = attn @ q (fp8 DoubleRow, contraction 256/matmul) + x

All batches (2 per body x repeat bodies) form one flat pipeline:
step t emits phase1(t) [MM1 + softmax + attnT] with phase2(t-1)
[MM2 + epilogue] blocks interleaved 1:1, so the drain-paced MM2 stream
rides inside the PE-dense MM1 stream and the in-order PE queue never
sits behind a PSUM-bank drain.  Input tiles are triple-buffered so batch
t+2's cast-DMA loads are emitted (and start) a full step ahead.

Epilogue alternates per block: even = DVE add-from-PSUM (+x, bf16 out);
odd = PE identity-matmul accumulates x into PSUM, ACT evac copy.
y is stored bf16 and upcast on the host.  Empirical rel-err of this
pipeline vs the f32 reference is ~8e-3 (tolerance 2e-2).
"""

import numpy as np

import concourse.bacc as bacc
import concourse.tile as tile
from concourse import mybir
from concourse.bass_utils import run_bass_kernel_spmd
from concourse.masks import make_identity

B, C, H, W = 16, 512, 64, 64
N = H * W            # 4096
NCORES = 8
BPC = B // NCORES    # batches per core
CP = C // 128        # 4 channel blocks
KP = N // 128        # 32 n-chunks
KPP = KP // 2        # 16 chunk-pairs
NJ = N // 512        # 8 output col blocks

f32 = mybir.dt.float32
bf16 = mybir.dt.bfloat16
f8 = mybir.dt.float8e4


def _build_pipeline(nc, tc, x, gamma, y, nbatch):
    pools = {}

    def pool(name, bufs, space="SBUF"):
        pools[name] = tc.alloc_tile_pool(name=name, bufs=bufs, space=space)
        return pools[name]

    qr_p = pool("qr", 3)                  # [128, N] bf16, tag per i
    q8_p = pool("q8", 3)                  # [128, 2, N] fp8, tag per half
    qt_p = pool("qt", 4)                  # [128, 2C] bf16 transient pairs
    attn_p = pool("attn", 4)              # [128, C] bf16 + fp8
    attnT_p = pool("attnT", 4)            # [128, 2, C] fp8 kd-pairs
    outsb_p = pool("outsb", 6)            # [128, 512] bf16
    small_p = pool("small", 8)            # [128, 1] f32
    const_p = pool("const", 1)
    ps_t = pool("ps_t", 2, space="PSUM")      # transpose staging banks
    ps_aff = pool("ps_aff", 1, space="PSUM")  # tag per i -> 4 banks
    ps_out = pool("ps_out", 2, space="PSUM")

    def load_q(t):
        # cast-load q c-block rows f32 -> bf16, then fp8 interleaved halves
        b = t % BPC
        q = {}
        for i in range(CP):
            qi = qr_p.tile([128, N], bf16, tag=f"qr{i}", name=f"qr{i}")
            nc.gpsimd.dma_start(
                out=qi, in_=x[b, 128 * i:128 * (i + 1), :])
            q[i] = qi
        q8 = {}
        for half in range(2):
            t8 = q8_p.tile([128, 2, N], f8, tag=f"q8{half}", name=f"q8{half}")
            for j in range(2):
                nc.gpsimd.dma_start(out=t8[:, j, :], in_=q[2 * half + j])
            q8[half] = t8
        return q, q8

    # identities (f32 recon / bf16 data transposes)
    ident_f = const_p.tile([128, 128], f32)
    make_identity(nc, ident_f)

    # HAM warmup: PE transposes of the identity keep the PE busy while
    # the first loads land, so the clock gate opens before real matmuls.
    warm_ps = ps_t.tile([128, C], f32, tag="pst", name="warm_ps")
    for w in range(16):
        nc.tensor.transpose(
            warm_ps[:, 128 * (w % CP):128 * (w % CP + 1)], ident_f, ident_f)
    warm_sb = const_p.tile([128, C], f32)
    nc.vector.tensor_copy(out=warm_sb, in_=warm_ps)

    ident = const_p.tile([128, 128], bf16)
    nc.gpsimd.dma_start(out=ident, in_=ident_f)
    gamma_sb = const_p.tile([128, 1], f32)
    nc.gpsimd.dma_start(out=gamma_sb, in_=gamma.to_broadcast([128, 1]))

    def phase1_gen(t, q, out_attnT):
        # MM1 software-pipelined (lookahead 2 chunk-pairs)
        qts = {}

        def emit_stage(p):
            pst = ps_t.tile([128, 2 * C], bf16, tag="pst", name="pst")
            for s in range(2):
                k = 2 * p + s
                for i in range(CP):
                    nc.tensor.transpose(
                        pst[:, C * s + 128 * i:C * s + 128 * (i + 1)],
                        q[i][:, 128 * k:128 * (k + 1)],
                        ident,
                    )
            qt = qt_p.tile([128, 2 * C], bf16, tag="qt", name="qt")
            if p % 4 == 1:
                nc.scalar.copy(out=qt, in_=pst)
            else:
                nc.vector.tensor_copy(out=qt, in_=pst)
            qts[p] = qt

        emit_stage(0)
        emit_stage(1)
        aff = [ps_aff.tile([128, C], f32, tag=f"aff{i}", name=f"aff{i}")
               for i in range(CP)]
        # upper block-triangle only; widths 512/384/256/128
        for k in range(KP):
            p, s = divmod(k, 2)
            if s == 1 and p + 2 < KPP:
                emit_stage(p + 2)
            qt = qts[p]
            base = C * s
            for i in range(CP):
                lo = 128 * i
                nc.tensor.matmul(
                    aff[i][:, lo:],
                    qt[:, base + 128 * i:base + 128 * (i + 1)],
                    qt[:, base + lo:base + C],
                    start=(k == 0),
                    stop=(k == KP - 1),
                )
            if s == 1:
                del qts[p]
            yield
        for (bi, bj) in [(1, 0), (2, 0), (2, 1), (3, 0), (3, 1), (3, 2)]:
            tmp = qt_p.tile([128, 128], f32, tag="tri", name="tri")
            nc.scalar.copy(
                out=tmp, in_=aff[bj][:, 128 * bi:128 * (bi + 1)])
            nc.tensor.matmul(
                aff[bi][:, 128 * bj:128 * (bj + 1)],
                tmp, ident_f, is_transpose=True, skip_group_check=True,
            )

        # softmax(min-centered, negated); gamma/Z prescale converts to fp8
        a8 = [None] * CP
        for i in (0, 1, 2, 3):
            m = small_p.tile([128, 1], f32, tag="m")
            nc.vector.tensor_reduce(
                out=m, in_=aff[i], op=mybir.AluOpType.min, axis=mybir.AxisListType.X
            )
            a_t = attn_p.tile([128, C], bf16, tag="a_t", name="a_t")
            z = small_p.tile([128, 1], f32, tag="z")
            nc.scalar.activation(
                out=a_t, in_=aff[i], func=mybir.ActivationFunctionType.Exp,
                bias=m, scale=-1.0, accum_out=z,
            )
            rz = small_p.tile([128, 1], f32, tag="rz")
            nc.vector.reciprocal(out=rz, in_=z)
            g = small_p.tile([128, 1], f32, tag="grz", name="grz")
            nc.vector.tensor_scalar_mul(out=g, in0=rz, scalar1=gamma_sb)
            a8i = attn_p.tile([128, C], bf16, tag="a8", name="a8")
            nc.vector.tensor_scalar_mul(out=a8i, in0=a_t, scalar1=g)
            a8[i] = a8i
            yield
        out_attnT.append(a8)

    def attnT_gen(t, a8, out_attnT):
        # attnT via PE transpose in bf16 (fp8 transpose mode is illegal:
        # walrus requires output element step 2); the evac copy converts
        # bf16 -> fp8.  Two [128, 2, 512] kd-pair tiles map 1:1 onto the
        # DoubleRow k-tile slices.  Emitted a few chunks into the NEXT
        # batch's MM1 so the in-order PE queue is not parked behind the
        # softmax chain.
        for u in range(2):
            pst = ps_t.tile([128, 2, C], bf16, tag="pst", name="pst8")
            for s in range(2):
                kd = 2 * u + s
                for i in range(CP):
                    nc.tensor.transpose(
                        pst[:, s, 128 * i:128 * (i + 1)],
                        a8[i][:, 128 * kd:128 * (kd + 1)],
                        ident,
                    )
            at8 = attnT_p.tile([128, 2, C], f8, tag="at", name="at")
            if u == 0:
                nc.scalar.copy(out=at8, in_=pst)
            else:
                nc.vector.tensor_copy(out=at8, in_=pst)
            out_attnT.append(at8)

    def phase2_gen(t, q, q8, attnT):
        # MM2 (fp8 DoubleRow, contraction 256/matmul) + epilogue.
        # Epilogue alternates per block between a direct DVE add-from-PSUM
        # and PE identity-matmul (+x into PSUM) + ACT evac copy, so drain
        # bandwidth comes from two engines.
        b = t % BPC
        for i in range(CP):
            oq = None
            for j in range(NJ):
                blk = i * NJ + j
                po = ps_out.tile([128, 512], f32, tag="po", name="po")
                pe_add = (blk % 2 == 1)
                for u in range(2):
                    nc.tensor.matmul(
                        po,
                        attnT[u][:, :, 128 * i:128 * (i + 1)],
                        q8[u][:, :, 512 * j:512 * (j + 1)],
                        start=(u == 0),
                        stop=(u == 1) and not pe_add,
                        perf_mode=mybir.MatmulPerfMode.DoubleRow,
                    )
                # epilogue lands in a quad-wide staging tile; one y DMA
                # per 4 blocks keeps the HWDGE ring off the critical path
                if j % 4 == 0:
                    oq = outsb_p.tile([128, 2048], bf16, tag="oq", name="oq")
                osl = oq[:, 512 * (j % 4):512 * (j % 4 + 1)]
                if pe_add:
                    nc.tensor.matmul(
                        po, ident, q[i][:, 512 * j:512 * (j + 1)],
                        start=False, stop=True,
                    )
                    nc.scalar.copy(out=osl, in_=po)
                else:
                    nc.vector.tensor_add(
                        out=osl, in0=po,
                        in1=q[i][:, 512 * j:512 * (j + 1)],
                    )
                if j % 4 == 3:
                    nc.sync.dma_start(
                        out=y[b, 128 * i:128 * (i + 1),
                              512 * (j - 3):512 * (j + 1)],
                        in_=oq,
                    )
                yield

    # ---- flat pipeline over all batches ----
    qs, q8s, attnTs = {}, {}, {}
    qs[0], q8s[0] = load_q(0)
    if nbatch > 1:
        qs[1], q8s[1] = load_q(1)
    # p1 yields: 32 chunks + 4 softmax rows.  attnT(t-1) is emitted 4
    # chunks into p1(t); p2(t-1) blocks are interleaved with extra blocks
    # reserved for the PE-light softmax tail.
    blocks_at = [0] * 8 + [1] * 24 + [2, 2, 2, 2]
    prev_p2 = None
    prev_tail = None
    for t in range(nbatch):
        attnTs[t] = []
        a8_box = []
        p1 = phase1_gen(t, qs[t], a8_box)
        for step, _ in enumerate(p1):
            if step == 4 and prev_tail is not None:
                attnT_gen(*prev_tail)
                prev_tail = None
            if prev_p2 is not None:
                for _ in range(blocks_at[step]):
                    next(prev_p2, None)
        if prev_p2 is not None:
            for _ in prev_p2:
                pass
        # prefetch batch t+2 (slot of batch t-1; its phase2 was fully
        # emitted during this step, so the tile-pool dependency is sound)
        if t + 2 < nbatch:
            qs[t + 2], q8s[t + 2] = load_q(t + 2)
            qs.pop(t - 1, None), q8s.pop(t - 1, None)
            attnTs.pop(t - 1, None)
        prev_tail = (t, a8_box[0], attnTs[t])
        prev_p2 = phase2_gen(t, qs[t], q8s[t], attnTs[t])
    attnT_gen(*prev_tail)
    for _ in prev_p2:
        pass

    for p in reversed(list(pools.values())):
        p.release()


_NC_CACHE = {}


def build_kernel(bpc=BPC, repeat=1):
    key = (bpc, repeat)
    if key in _NC_CACHE:
        return _NC_CACHE[key]
    global BPC
    old_bpc, BPC = BPC, bpc
    try:
        nc = bacc.Bacc("TRN2", target_bir_lowering=False, debug=False, num_devices=1)
        x = nc.dram_tensor("x", [bpc, C, N], f32, kind="ExternalInput").ap()
        gamma = nc.dram_tensor("gamma", [1], f32, kind="ExternalInput").ap()
        y = nc.dram_tensor("y", [bpc, C, N], bf16, kind="ExternalOutput").ap()
        with tile.TileContext(nc) as tc:
            _build_pipeline(nc, tc, x, gamma, y, nbatch=bpc * repeat)
        nc.compile()
    finally:
        BPC = old_bpc
    _NC_CACHE[key] = nc
    return nc


def run(x, gamma, trace=False):
    """x: [B, C, H, W] f32, gamma: [1] f32 -> ([B, C, H, W] f32, results)"""
    x = np.ascontiguousarray(x, dtype=np.float32).reshape(B, C, N)
    gamma = np.ascontiguousarray(gamma, dtype=np.float32)
    nc = build_kernel()
    in_maps = [
        {"x": x[i * BPC:(i + 1) * BPC], "gamma": gamma} for i in range(NCORES)
    ]
    res = run_bass_kernel_spmd(nc, in_maps, core_ids=list(range(NCORES)),
                               trace=trace)
    out = np.concatenate([res.results[i]["y"] for i in range(NCORES)], axis=0)
    return out.reshape(B, C, H, W).astype(np.float32), res


def kernel(x, gamma):
    out, _ = run(x, gamma)
    return out
